# revision 1
# baseline (speedup 1.0000x reference)
"""DCRNN (PEMS-BAY) Trainium2 Bass kernel, data-parallel over batch on 8 cores.

Layouts per core (local batch BL=8):
  A-layout: [feature partitions, b*384 + n]  (n padded 325->384; 8*384 = 3072 cols)
  B-layout: [node-chunk partitions (128/128/69), b*Fout + f]
gconv (W-first):  out = X@A0 + S @ (X@W1 + S @ (X@(2*W2)))
  P2,P1 = W-matmuls in A-layout; transpose down to B; X1 = S@P2B; Q = X1+P1B;
  R = S@QB; PE-transposes of R accumulate onto the X@A0 PSUM banks; activation
  evacuates PSUM -> SBUF.
State tile XH per layer: rows 0:64 = h, rows 64:128 = x (padded features).
"""
import sys
import os
import numpy as np

sys.path.insert(0, "/opt/trn_rl_repo")

import concourse.bass as bass  # noqa: E402
import concourse.mybir as mybir  # noqa: E402
import concourse.tile as tile  # noqa: E402
from concourse import bacc  # noqa: E402
from concourse.bass_utils import run_bass_kernel_spmd  # noqa: E402
from concourse.masks import make_identity  # noqa: E402

# problem constants
N = 325
B = 64
T = 12
HZ = 12
U = 64
DIN = 2
DOUT = 1
NCORES = 8
BL = B // NCORES          # 8 local batch
NB = 384                  # padded node stride per batch
AF = BL * NB              # 3072 A-layout free width
NCH = [(0, 128), (128, 128), (256, 69)]   # node chunks (offset, len)
NBANK = AF // 512         # 6 psum banks for a full A row

F32 = mybir.dt.float32
MMDT = mybir.dt.float32r  # matmul input dtype (float32 | float32r)
AFT = mybir.ActivationFunctionType

CELLS = ["enc0", "enc1", "dec0", "dec1"]
CELL_DIN = {"enc0": DIN, "enc1": U, "dec0": DOUT, "dec1": U}

_BUILD_CACHE = {}
LAST_RESULT = None


def _install_ntff_hook():
    """Register the axon NTFF profiling hook if the image lacks antenv.axon_hooks."""
    import types
    import antenv
    if getattr(antenv, "axon_hooks", None) is not None:
        return
    m = types.ModuleType("antenv.axon_hooks")
    state = {"h": None}
    m.set_axon_ntff_profile_hook = lambda h: state.__setitem__("h", h)
    m.get_axon_ntff_profile_hook = lambda: state["h"]
    sys.modules["antenv.axon_hooks"] = m
    antenv.axon_hooks = m
    try:
        from trn_agent_boot.trn_boot import _ntff_profile_via_ctypes
        hook = _ntff_profile_via_ctypes("/opt/axon/libaxon_pjrt.so")
        if hook is not None:
            m.set_axon_ntff_profile_hook(hook)
    except Exception:
        pass


def _pad_w(w, din, fout):
    """(3F, fout) -> three [128, fout] padded mats A0, W1, 2*W2.

    Padded row map: rows 0:64 <- h/rh features (orig rows din:F),
    rows 64:64+din <- x features (orig rows 0:din). Others zero.
    """
    f = din + U
    w0, w1, w2 = w[0:f], w[f:2 * f], w[2 * f:3 * f]

    def pad(m):
        p = np.zeros((128, fout), np.float32)
        p[0:64] = m[din:f]
        p[64:64 + din] = m[0:din]
        return p

    return pad(w0 - w2), pad(w1), pad(2.0 * w2)


def _build(nsteps_enc, nsteps_dec):
    key = (nsteps_enc, nsteps_dec)
    if key in _BUILD_CACHE:
        return _BUILD_CACHE[key]

    nc = bacc.Bacc()
    # ---- DRAM params ----
    x_in = nc.declare_dram_parameter("x", [T, DIN, AF], MMDT, isOutput=False)
    s_in = nc.declare_dram_parameter("s", [N, N], MMDT, isOutput=False)
    wparams = {}
    for c in CELLS:
        for nm, shp in [("gA0", [128, 128]), ("gW1", [128, 128]),
                        ("gW2", [128, 128]), ("cA0", [128, 64]),
                        ("cW1", [128, 64]), ("cW2", [128, 64]),
                        ("gb", [128, 1]), ("cb", [64, 1])]:
            dt_ = F32 if nm in ("gb", "cb") else MMDT
            wparams[f"{c}_{nm}"] = nc.declare_dram_parameter(
                f"{c}_{nm}", shp, dt_, isOutput=False)
    wparams["pW"] = nc.declare_dram_parameter("pW", [64, 1], MMDT, isOutput=False)
    wparams["pb"] = nc.declare_dram_parameter("pb", [1, 1], F32, isOutput=False)
    out_d = nc.declare_dram_parameter("out", [HZ, 1, BL, N], F32, isOutput=True)

    with tile.TileContext(nc) as tc:
        with tc.tile_pool(name="const", bufs=1) as cp, \
             tc.tile_pool(name="state", bufs=1) as st, \
             tc.tile_pool(name="pa", bufs=1) as pa, \
             tc.tile_pool(name="bp", bufs=1) as bp, \
             tc.tile_pool(name="pstagep", bufs=6, space="PSUM") as psp:

            # ---- constants to SBUF ----
            wt = {}
            for c in CELLS:
                for nm in ["gA0", "gW1", "gW2"]:
                    wt[f"{c}_{nm}"] = cp.tile([128, 128], MMDT, tag=f"{c}_{nm}", name=f"{c}_{nm}")
                for nm in ["cA0", "cW1", "cW2"]:
                    wt[f"{c}_{nm}"] = cp.tile([128, 64], MMDT, tag=f"{c}_{nm}", name=f"{c}_{nm}")
                wt[f"{c}_gb"] = cp.tile([128, 1], F32, tag=f"{c}_gb", name=f"{c}_gb")
                wt[f"{c}_cb"] = cp.tile([64, 1], F32, tag=f"{c}_cb", name=f"{c}_cb")
            wt["pW"] = cp.tile([64, 1], MMDT, tag="pW", name="pW")
            wt["pb"] = cp.tile([1, 1], F32, tag="pb", name="pb")
            for k, t in wt.items():
                nc.sync.dma_start(out=t, in_=wparams[k][:])
            s_t = []
            for ci, (c0, cl) in enumerate(NCH):
                stl = cp.tile([128, N], MMDT, tag=f"s{ci}", name=f"s{ci}")
                nc.sync.dma_start(out=stl[0:cl, :], in_=s_in[c0:c0 + cl, :])
                s_t.append(stl)
            ident = cp.tile([128, 128], F32, tag="ident")
            make_identity(nc, ident)

            # ---- state tiles (split per batch-half for pipeline overlap) ----
            HB = BL // 2          # 4 batches per half
            HAF = HB * NB         # 1536 A-cols per half
            xh = {}
            xrh, r_h, u_h, c_h = {}, {}, {}, {}
            for hf in range(2):
                for c in CELLS:
                    xh[(c, hf)] = st.tile([128, HAF], MMDT, tag=f"xh_{c}_{hf}",
                                          name=f"xh_{c}_{hf}")
                xrh[hf] = st.tile([128, HAF], MMDT, tag=f"xr{hf}", name=f"xr{hf}")
                r_h[hf] = st.tile([64, HAF], F32, tag=f"r{hf}", name=f"r{hf}")
                u_h[hf] = st.tile([64, HAF], F32, tag=f"u{hf}", name=f"u{hf}")
                c_h[hf] = st.tile([64, HAF], F32, tag=f"c{hf}", name=f"c{hf}")

            for tl in [xh[k] for k in xh] + [xrh[0], xrh[1]]:
                nc.vector.memset(tl[:, :].bitcast(F32), 0.0)
            tc.strict_bb_all_engine_barrier()

            def gconv(cell, rhs_t, wprefix, fout, hf):
                """One gconv on one batch-half. rhs_t: [128, HAF] MMDT tile.
                Returns preact psum tile [128, HAF] (rows 0:fout valid)."""
                a0w = wt[f"{cell}_{wprefix}A0"]
                w1 = wt[f"{cell}_{wprefix}W1"]
                w2 = wt[f"{cell}_{wprefix}W2"]
                fhh = HB * fout           # B free width per half (512/256)

                p2a = pa.tile([128, HAF], F32, tag=f"p2a{hf}", name=f"p2a{hf}")
                p1a = pa.tile([128, HAF], F32, tag=f"p1a{hf}", name=f"p1a{hf}")
                for w_, dst in ((w2, p2a), (w1, p1a)):
                    for ci in range(3):
                        sl = slice(ci * 512, (ci + 1) * 512)
                        pt = psp.tile([128, 512], F32, tag="pstage")
                        nc.tensor.matmul(pt[0:fout, :], w_[0:128, :],
                                         rhs_t[:, sl], start=True, stop=True)
                        nc.scalar.copy(dst[0:fout, sl], pt[0:fout, :])

                # P0 -> SBUF
                p0a = pa.tile([128, HAF], F32, tag=f"p0a{hf}", name=f"p0a{hf}")
                for ci in range(3):
                    sl = slice(ci * 512, (ci + 1) * 512)
                    pt = psp.tile([128, 512], F32, tag="pstage")
                    nc.tensor.matmul(pt[0:fout, :], a0w[0:128, :],
                                     rhs_t[:, sl], start=True, stop=True)
                    nc.scalar.copy(p0a[0:fout, sl], pt[0:fout, :])

                # down-transposes P2A,P1A -> B-layout
                bt = {}
                for role, srct in (("p2b", p2a), ("p1b", p1a)):
                    dt_ = MMDT if role == "p2b" else F32
                    tiles = [bp.tile([128, 512], dt_, tag=f"{role}{ci}_{hf}",
                                     name=f"{role}{ci}_{hf}") for ci in range(3)]
                    for ci, (c0, cl) in enumerate(NCH):
                        dt = psp.tile([128, 512], F32, tag="pstage")
                        for j in range(HB):
                            nc.tensor.matmul(
                                dt[0:cl, j * fout:(j + 1) * fout],
                                srct[0:fout, j * NB + c0: j * NB + c0 + cl],
                                ident[0:fout, 0:fout], is_transpose=True,
                                start=(j == 0), stop=(j == HB - 1))
                        if role == "p2b":
                            nc.vector.tensor_copy(tiles[ci][0:cl, 0:fhh],
                                                  dt[0:cl, 0:fhh])
                        else:
                            nc.scalar.copy(tiles[ci][0:cl, 0:fhh],
                                           dt[0:cl, 0:fhh])
                    bt[role] = tiles

                # X1 = S@P2B ; Q = X1 + P1B ; R = S@QB
                qb = [bp.tile([128, 512], MMDT, tag=f"p2b{ci}_{hf}",
                              name=f"qb{ci}_{hf}") for ci in range(3)]
                rbt = [bp.tile([128, 512], F32, tag=f"p1b{ci}_{hf}",
                               name=f"rb{ci}_{hf}") for ci in range(3)]
                for dst, srcs, srcadd in ((qb, bt["p2b"], bt["p1b"]),
                                          (rbt, qb, None)):
                    for mi, (m0, ml) in enumerate(NCH):
                        xt = psp.tile([128, 512], F32, tag="pstage")
                        for ki, (k0, kl) in enumerate(NCH):
                            nc.tensor.matmul(
                                xt[0:ml, 0:fhh], s_t[ki][0:kl, m0:m0 + ml],
                                srcs[ki][0:kl, 0:fhh],
                                start=(ki == 0), stop=(ki == 2))
                        if srcadd is not None:
                            nc.vector.tensor_tensor(
                                dst[mi][0:ml, 0:fhh], xt[0:ml, 0:fhh],
                                srcadd[mi][0:ml, 0:fhh], mybir.AluOpType.add)
                        else:
                            nc.scalar.copy(dst[mi][0:ml, 0:fhh],
                                           xt[0:ml, 0:fhh])

                # up-transposes RB -> psum bank; add P0A -> SBUF preact
                preact = pa.tile([128, HAF], F32, tag=f"p2a{hf}",
                                 name=f"pre{hf}")
                blocks_by_bank = {}
                for b in range(HB):
                    for ci in range(3):
                        c0, cl = NCH[ci]
                        blocks_by_bank.setdefault((b * NB + c0) // 512,
                                                  []).append((b, ci))
                for bk, blks in blocks_by_bank.items():
                    ut = psp.tile([128, 512], F32, tag="pstage")
                    for j, (b, ci) in enumerate(blks):
                        c0, cl = NCH[ci]
                        off = b * NB + c0 - bk * 512
                        nc.tensor.matmul(
                            ut[0:fout, off:off + cl],
                            rbt[ci][0:cl, b * fout:(b + 1) * fout],
                            ident[0:cl, 0:cl], is_transpose=True,
                            start=(j == 0), stop=(j == len(blks) - 1))
                    sl = slice(bk * 512, (bk + 1) * 512)
                    nc.vector.tensor_tensor(preact[0:fout, sl],
                                            ut[0:fout, :], p0a[0:fout, sl],
                                            mybir.AluOpType.add)
                return preact

            def cell(cname, hf, xh_t, xh_next):
                """DCGRU cell on one batch-half. x rows 64:128, h rows 0:64."""
                din = CELL_DIN[cname]
                xr = xrh[hf]
                r_t, u_t, c_t = r_h[hf], u_h[hf], c_h[hf]
                pre_g = gconv(cname, xh_t, "g", 128, hf)
                gb = wt[f"{cname}_gb"]
                for ci in range(3):
                    sl = slice(ci * 512, (ci + 1) * 512)
                    nc.scalar.activation(r_t[:, sl], pre_g[0:64, sl],
                                         AFT.Sigmoid, bias=gb[0:64, 0:1])
                    nc.scalar.activation(u_t[:, sl], pre_g[64:128, sl],
                                         AFT.Sigmoid, bias=gb[64:128, 0:1])
                nc.vector.tensor_tensor(xr[0:64, :], r_t[:, :], xh_t[0:64, :],
                                        mybir.AluOpType.mult)
                nc.vector.tensor_copy(xr[64:64 + din, :],
                                      xh_t[64:64 + din, :])
                pre_c = gconv(cname, xr, "c", 64, hf)
                cb = wt[f"{cname}_cb"]
                for ci in range(3):
                    sl = slice(ci * 512, (ci + 1) * 512)
                    nc.scalar.activation(c_t[:, sl], pre_c[0:64, sl],
                                         AFT.Tanh, bias=cb[0:64, 0:1])
                # h' = c + u*(h-c); r_t is dead, reuse it for temps
                nc.vector.tensor_tensor(r_t[:, :], xh_t[0:64, :], c_t[:, :],
                                        mybir.AluOpType.subtract)
                nc.vector.tensor_tensor(r_t[:, :], u_t[:, :], r_t[:, :],
                                        mybir.AluOpType.mult)
                nc.vector.tensor_tensor(xh_t[0:64, :], c_t[:, :], r_t[:, :],
                                        mybir.AluOpType.add)
                if xh_next is not None:
                    nc.vector.tensor_copy(xh_next[64:128, :], xh_t[0:64, :])

            # ---- encoder ----
            xr3 = x_in[:].rearrange("t d (g f) -> t d g f", g=2)
            for t in range(nsteps_enc):
                for hf in range(2):
                    nc.sync.dma_start(out=xh[("enc0", hf)][64:66, :],
                                      in_=xr3[t, :, hf, :])
                for hf in range(2):
                    cell("enc0", hf, xh[("enc0", hf)], xh[("enc1", hf)])
                    cell("enc1", hf, xh[("enc1", hf)], None)

            # ---- copy encoder state to decoder ----
            for hf in range(2):
                nc.gpsimd.tensor_copy(xh[("dec0", hf)][0:64, :],
                                      xh[("enc0", hf)][0:64, :])
                nc.gpsimd.tensor_copy(xh[("dec1", hf)][0:64, :],
                                      xh[("enc1", hf)][0:64, :])

            # ---- decoder ----
            for t in range(nsteps_dec):
                for hf in range(2):
                    cell("dec0", hf, xh[("dec0", hf)], xh[("dec1", hf)])
                    cell("dec1", hf, xh[("dec1", hf)], None)
                    for ci in range(3):
                        sl = slice(ci * 512, (ci + 1) * 512)
                        pt = psp.tile([128, 512], F32, tag="pstage")
                        nc.tensor.matmul(pt[0:1, :], wt["pW"][0:64, :],
                                         xh[("dec1", hf)][0:64, sl],
                                         start=True, stop=True)
                        nc.scalar.activation(xh[("dec0", hf)][64:65, sl],
                                             pt[0:1, :], AFT.Identity,
                                             bias=wt["pb"][0:1, 0:1])
                    ov = xh[("dec0", hf)][64:65, :].bitcast(F32).rearrange(
                        "p (b n) -> p b n", b=HB)
                    nc.sync.dma_start(out=out_d[t][:, hf * HB:(hf + 1) * HB, :],
                                      in_=ov[:, :, 0:N])

    nc.finalize()
    _BUILD_CACHE[key] = nc
    return nc


def _prep_inputs(inputs, support, weights):
    """Host-side prep. Returns (shared_map, per_core_x list)."""
    shared = {"s": np.ascontiguousarray(support, np.float32)}
    for c in CELLS:
        din = CELL_DIN[c]
        ga0, gw1, gw2 = _pad_w(weights[f"{c}_gate_W"], din, 2 * U)
        ca0, cw1, cw2 = _pad_w(weights[f"{c}_cand_W"], din, U)
        gb = np.zeros((128, 1), np.float32)
        gb[:, 0] = weights[f"{c}_gate_b"]
        cb = np.zeros((64, 1), np.float32)
        cb[:, 0] = weights[f"{c}_cand_b"]
        shared.update({f"{c}_gA0": ga0, f"{c}_gW1": gw1, f"{c}_gW2": gw2,
                       f"{c}_cA0": ca0, f"{c}_cW1": cw1, f"{c}_cW2": cw2,
                       f"{c}_gb": gb, f"{c}_cb": cb})
    shared["pW"] = np.ascontiguousarray(weights["proj_W"], np.float32)
    shared["pb"] = np.asarray(weights["proj_b"], np.float32).reshape(1, 1)

    # inputs (T, B, N*DIN) -> per-core (T, DIN, AF) with node padding
    x = np.asarray(inputs, np.float32).reshape(T, B, N, DIN)
    per_core = []
    for c in range(NCORES):
        xc = x[:, c * BL:(c + 1) * BL]                  # (T, BL, N, DIN)
        xp = np.zeros((T, DIN, BL, NB), np.float32)
        xp[:, :, :, 0:N] = xc.transpose(0, 3, 1, 2)
        per_core.append(xp.reshape(T, DIN, AF))
    return shared, per_core


def kernel(**inputs) -> np.ndarray:
    support = np.asarray(inputs["support"], np.float32)
    weights = {k: np.asarray(v, np.float32) for k, v in inputs.items()
               if k not in ("inputs", "support")}
    shared, per_core_x = _prep_inputs(inputs["inputs"], support, weights)

    nc = _build(T, HZ)
    if os.environ.get("DCRNN_TRACE"):
        _install_ntff_hook()
    in_maps = [dict(shared, x=per_core_x[c]) for c in range(NCORES)]
    res = run_bass_kernel_spmd(nc, in_maps, list(range(NCORES)),
                               trace=bool(os.environ.get("DCRNN_TRACE")))
    global LAST_RESULT
    LAST_RESULT = res
    if res.exec_time_ns is not None:
        print(f"HW exec time: {res.exec_time_ns} ns")
    outs = [res.results[c]["out"].reshape(HZ, BL, N) for c in range(NCORES)]
    return np.concatenate(outs, axis=1).astype(np.float32)


if __name__ == "__main__":
    sys.path.insert(0, "/root/problem")
    import reference
    ins = reference.setup_inputs()
    ins = {k: np.asarray(v) for k, v in ins.items()}
    exp = np.asarray(reference.reference(**ins))
    act = kernel(**ins)
    err = np.max(np.abs(act - exp)) / (np.abs(exp).max() + 1e-30)
    print("Relative error:", err)



# revision 5
# speedup vs baseline: 2.6093x; 2.6093x over previous
"""DCRNN (PEMS-BAY) Trainium2 Bass kernel, data-parallel over batch on 8 cores.

Transposeless formulation, all matmul inputs bf16 (PSUM accumulates fp32):
  gconv(X) = X@A0 + S@(X@W1) + S2@(X@2W2),  A0 = W0 - W2, S2 = S@S.
Per batch j (4 per half):
  down:  P_j = (X_Aslab_j)^T @ [W1|2W2]   -> B-layout P1|P2 in PSUM -> SBUF bf16
  up:    pre_j = A0^T @ X_j + sum_k P1_j[k,:]^T @ S[k,:] + P2_j[k,:]^T @ S2[k,:]
         (7 accumulating matmuls into one PSUM bank, A-layout [fout, 325])
  act:   sigmoid/tanh directly from PSUM with per-partition bias.
State tiles XH per cell/half: rows 0:64 = h, rows 64:64+din = x (A-layout,
cols = b*384 + n).  Decoder reuses encoder state tiles (no handoff copies).
"""
import sys
import os
import numpy as np

sys.path.insert(0, "/opt/trn_rl_repo")

import concourse.bass as bass  # noqa: E402
import concourse.mybir as mybir  # noqa: E402
import concourse.tile as tile  # noqa: E402
from concourse import bacc  # noqa: E402
from concourse.bass_utils import run_bass_kernel_spmd  # noqa: E402

import ml_dtypes  # noqa: E402

# problem constants
N = 325
B = 64
T = 12
HZ = 12
U = 64
DIN = 2
DOUT = 1
NCORES = 8
BL = B // NCORES          # 8 local batch
NB = 384                  # padded node stride per batch
AF = BL * NB              # 3072 A-layout free width
NCH = [(0, 128), (128, 128), (256, 69)]   # node chunks (offset, len)
HB = BL // 2              # 4 batches per half
HAF = HB * NB             # 1536 A-cols per half

F32 = mybir.dt.float32
BF16 = mybir.dt.bfloat16
AFT = mybir.ActivationFunctionType
ALU = mybir.AluOpType
BF16NP = np.dtype(ml_dtypes.bfloat16)

CELLS = ["enc0", "enc1", "dec0", "dec1"]
CELL_DIN = {"enc0": DIN, "enc1": U, "dec0": DOUT, "dec1": U}

_BUILD_CACHE = {}
LAST_RESULT = None


def _install_ntff_hook():
    """Register the axon NTFF profiling hook if the image lacks antenv.axon_hooks."""
    import types
    import antenv
    if getattr(antenv, "axon_hooks", None) is not None:
        return
    m = types.ModuleType("antenv.axon_hooks")
    state = {"h": None}
    m.set_axon_ntff_profile_hook = lambda h: state.__setitem__("h", h)
    m.get_axon_ntff_profile_hook = lambda: state["h"]
    sys.modules["antenv.axon_hooks"] = m
    antenv.axon_hooks = m
    try:
        from trn_agent_boot.trn_boot import _ntff_profile_via_ctypes
        hook = _ntff_profile_via_ctypes("/opt/axon/libaxon_pjrt.so")
        if hook is not None:
            m.set_axon_ntff_profile_hook(hook)
    except Exception:
        pass


def _pad_w(w, din, fout):
    """(3F, fout) -> padded [128, fout] mats A0=W0-W2, W1, 2*W2.

    Padded row map: rows 0:64 <- h features (orig rows din:F),
    rows 64:64+din <- x features (orig rows 0:din). Others zero.
    """
    f = din + U
    w0, w1, w2 = w[0:f], w[f:2 * f], w[2 * f:3 * f]

    def pad(m):
        p = np.zeros((128, fout), np.float32)
        p[0:64] = m[din:f]
        p[64:64 + din] = m[0:din]
        return p

    return pad(w0 - w2), pad(w1), pad(2.0 * w2)


def _build(nsteps_enc, nsteps_dec):
    key = (nsteps_enc, nsteps_dec)
    if key in _BUILD_CACHE:
        return _BUILD_CACHE[key]

    nc = bacc.Bacc()
    # ---- DRAM params ----
    x_in = nc.declare_dram_parameter("x", [T, DIN, AF], BF16, isOutput=False)
    s_in = nc.declare_dram_parameter("s", [N, N], BF16, isOutput=False)
    s2_in = nc.declare_dram_parameter("s2", [N, N], BF16, isOutput=False)
    wparams = {}
    for c in CELLS:
        for nm, shp, dt_ in [("gA0", [128, 128], BF16), ("gW12", [128, 256], BF16),
                             ("cA0", [128, 64], BF16), ("cW12", [128, 128], BF16),
                             ("gb", [128, 1], F32), ("cb", [64, 1], F32)]:
            wparams[f"{c}_{nm}"] = nc.declare_dram_parameter(
                f"{c}_{nm}", shp, dt_, isOutput=False)
    wparams["pW"] = nc.declare_dram_parameter("pW", [64, 1], BF16, isOutput=False)
    wparams["pb"] = nc.declare_dram_parameter("pb", [1, 1], F32, isOutput=False)
    out_d = nc.declare_dram_parameter("out", [HZ, 1, BL, N], BF16, isOutput=True)

    with tile.TileContext(nc) as tc:
        with tc.tile_pool(name="const", bufs=1) as cp, \
             tc.tile_pool(name="state", bufs=1) as st, \
             tc.tile_pool(name="bp", bufs=2) as bp, \
             tc.tile_pool(name="pdp", bufs=4, space="PSUM") as pdp, \
             tc.tile_pool(name="pup", bufs=4, space="PSUM") as pup:

            # ---- constants to SBUF ----
            wt = {}
            for c in CELLS:
                for nm, shp in [("gA0", [128, 128]), ("gW12", [128, 256]),
                                ("cA0", [128, 64]), ("cW12", [128, 128])]:
                    wt[f"{c}_{nm}"] = cp.tile(shp, BF16, tag=f"{c}_{nm}",
                                              name=f"{c}_{nm}")
                wt[f"{c}_gb"] = cp.tile([128, 1], F32, tag=f"{c}_gb", name=f"{c}_gb")
                wt[f"{c}_cb"] = cp.tile([64, 1], F32, tag=f"{c}_cb", name=f"{c}_cb")
            wt["pW"] = cp.tile([64, 1], BF16, tag="pW", name="pW")
            wt["pb"] = cp.tile([1, 1], F32, tag="pb", name="pb")
            for k, t in wt.items():
                nc.sync.dma_start(out=t, in_=wparams[k][:])
            s_t, s2_t = [], []
            for ci, (c0, cl) in enumerate(NCH):
                stl = cp.tile([128, N], BF16, tag=f"s{ci}", name=f"s{ci}")
                nc.sync.dma_start(out=stl[0:cl, :], in_=s_in[c0:c0 + cl, :])
                s_t.append(stl)
                stl2 = cp.tile([128, N], BF16, tag=f"t{ci}", name=f"t{ci}")
                nc.sync.dma_start(out=stl2[0:cl, :], in_=s2_in[c0:c0 + cl, :])
                s2_t.append(stl2)

            # ---- state tiles ----
            xh = {}
            xr_h, ru_h, cc_h, u0_h = {}, {}, {}, {}
            for hf in range(2):
                for c in ["enc0", "enc1"]:
                    xh[(c, hf)] = st.tile([128, HAF], BF16, tag=f"xh_{c}_{hf}",
                                          name=f"xh_{c}_{hf}")
                xr_h[hf] = st.tile([128, HAF], BF16, tag=f"xr{hf}", name=f"xr{hf}")
                ru_h[hf] = st.tile([128, HAF], BF16, tag=f"ru{hf}", name=f"ru{hf}")
                cc_h[hf] = st.tile([64, HAF], BF16, tag=f"cc{hf}", name=f"cc{hf}")
                u0_h[hf] = st.tile([64, HAF], BF16, tag=f"u0{hf}", name=f"u0{hf}")

            for tl in ([xh[k] for k in xh]
                       + [xr_h[0], xr_h[1], ru_h[0], ru_h[1], cc_h[0], cc_h[1],
                          u0_h[0], u0_h[1]]):
                nc.vector.memset(tl[:, :].bitcast(F32), 0.0)
            tc.strict_bb_all_engine_barrier()

            evac_flip = [0]

            def gconv(cell, src, pref, fout, hf, act_fn):
                """One gconv on one batch-half from A-layout src [128, HAF].
                act_fn(j, pu) must consume pu[0:fout, 0:N] (A-layout preact
                for local batch j, bias NOT yet applied)."""
                w12 = wt[f"{cell}_{pref}W12"]
                a0 = wt[f"{cell}_{pref}A0"]
                w2 = 2 * fout
                per = 512 // w2          # batches per psum bank: gate 2, cand 4
                pbs = []
                for ci, (c0, cl) in enumerate(NCH):
                    pb = bp.tile([128, HB * w2], BF16, tag=f"pb{pref}{hf}_{ci}",
                                 name=f"pb{pref}{hf}_{ci}")
                    for g0 in range(0, HB, per):
                        js = list(range(g0, min(g0 + per, HB)))
                        pd = pdp.tile([128, 512], F32, tag="pd")
                        for idx, j in enumerate(js):
                            nc.tensor.matmul(
                                pd[0:cl, idx * w2:(idx + 1) * w2],
                                src[0:128, j * NB + c0: j * NB + c0 + cl],
                                w12[0:128, 0:w2],
                                start=(idx == 0), stop=(idx == len(js) - 1))
                        lo = g0 * w2
                        span = len(js) * w2
                        if evac_flip[0] % 2 == 0:
                            nc.scalar.copy(pb[0:cl, lo:lo + span],
                                           pd[0:cl, 0:span])
                        else:
                            nc.vector.tensor_copy(pb[0:cl, lo:lo + span],
                                                  pd[0:cl, 0:span])
                        evac_flip[0] += 1
                    pbs.append(pb)
                for j in range(HB):
                    pu = pup.tile([128, 512], F32, tag="pu")
                    nc.tensor.matmul(pu[0:fout, 0:N], a0[0:128, 0:fout],
                                     src[0:128, j * NB: j * NB + N],
                                     start=True, stop=False)
                    for ci, (c0, cl) in enumerate(NCH):
                        nc.tensor.matmul(pu[0:fout, 0:N],
                                         pbs[ci][0:cl, j * w2: j * w2 + fout],
                                         s_t[ci][0:cl, 0:N],
                                         start=False, stop=False)
                        nc.tensor.matmul(pu[0:fout, 0:N],
                                         pbs[ci][0:cl, j * w2 + fout: j * w2 + w2],
                                         s2_t[ci][0:cl, 0:N],
                                         start=False, stop=(ci == 2))
                    act_fn(j, pu)

            def cell(cname, hf, xh_t, nxt_xh):
                """DCGRU cell on one batch-half. x rows 64:64+din, h rows 0:64."""
                xr_t, ru_t, cc_t = xr_h[hf], ru_h[hf], cc_h[hf]
                gb = wt[f"{cname}_gb"]
                cb = wt[f"{cname}_cb"]

                def gate_act(j, pu):
                    nc.scalar.activation(ru_t[0:128, j * NB: j * NB + N],
                                         pu[0:128, 0:N], AFT.Sigmoid,
                                         bias=gb[0:128, 0:1])

                def cand_act(j, pu):
                    nc.scalar.activation(cc_t[0:64, j * NB: j * NB + N],
                                         pu[0:64, 0:N], AFT.Tanh,
                                         bias=cb[0:64, 0:1])

                gconv(cname, xh_t, "g", 128, hf, gate_act)
                nc.vector.tensor_tensor(xr_t[0:64, :], ru_t[0:64, :],
                                        xh_t[0:64, :], ALU.mult)
                # u lives at partitions 64:128; DVE tensor_tensor needs equal
                # SBUF base partitions, so stage it at base 0 first
                nc.vector.tensor_copy(u0_h[hf][0:64, :], ru_t[64:128, :])
                gconv(cname, xr_t, "c", 64, hf, cand_act)
                # h' = c + u*(h-c); ru rows 0:64 (r) are dead, reuse as tmp
                nc.vector.tensor_tensor(ru_t[0:64, :], xh_t[0:64, :],
                                        cc_t[0:64, :], ALU.subtract)
                nc.vector.tensor_tensor(ru_t[0:64, :], u0_h[hf][0:64, :],
                                        ru_t[0:64, :], ALU.mult)
                nc.vector.tensor_tensor(xh_t[0:64, :], cc_t[0:64, :],
                                        ru_t[0:64, :], ALU.add)
                if nxt_xh is not None:
                    nc.vector.tensor_copy(nxt_xh[64:128, :], xh_t[0:64, :])
                    nc.vector.tensor_copy(xr_t[64:128, :], xh_t[0:64, :])

            # ---- encoder ----
            xd = x_in[:]
            for t in range(nsteps_enc):
                for hf in range(2):
                    sl = slice(hf * HAF, (hf + 1) * HAF)
                    nc.sync.dma_start(out=xh[("enc0", hf)][64:64 + DIN, :],
                                      in_=xd[t, :, sl])
                    nc.sync.dma_start(out=xr_h[hf][64:64 + DIN, :],
                                      in_=xd[t, :, sl])
                for hf in range(2):
                    cell("enc0", hf, xh[("enc0", hf)], xh[("enc1", hf)])
                for hf in range(2):
                    cell("enc1", hf, xh[("enc1", hf)], None)

            # ---- decoder (reuses encoder state tiles; GO symbol = 0) ----
            for hf in range(2):
                nc.vector.memset(xh[("enc0", hf)][64:65, :].bitcast(F32), 0.0)
                nc.vector.memset(xr_h[hf][64:65, :].bitcast(F32), 0.0)

            for t in range(nsteps_dec):
                for hf in range(2):
                    cell("dec0", hf, xh[("enc0", hf)], xh[("enc1", hf)])
                for hf in range(2):
                    cell("dec1", hf, xh[("enc1", hf)], None)
                    h1 = xh[("enc1", hf)]
                    x0 = xh[("enc0", hf)]
                    for ci in range(3):
                        pu = pup.tile([128, 512], F32, tag="pu")
                        nc.tensor.matmul(pu[0:1, 0:512], wt["pW"][0:64, 0:1],
                                         h1[0:64, ci * 512:(ci + 1) * 512],
                                         start=True, stop=True)
                        nc.scalar.activation(x0[64:65, ci * 512:(ci + 1) * 512],
                                             pu[0:1, 0:512], AFT.Identity,
                                             bias=wt["pb"][0:1, 0:1])
                    nc.vector.tensor_copy(xr_h[hf][64:65, :], x0[64:65, :])
                    ov = x0[64:65, :].rearrange("p (b n) -> p b n", b=HB)
                    nc.sync.dma_start(out=out_d[t][:, hf * HB:(hf + 1) * HB, :],
                                      in_=ov[:, :, 0:N])

    nc.finalize()
    _BUILD_CACHE[key] = nc
    return nc


def _prep_inputs(inputs, support, weights):
    """Host-side prep. Returns (shared_map, per_core_x list)."""
    s32 = np.asarray(support, np.float32)
    s2 = s32 @ s32
    # matmuls contract as sum_k M[k, m] * X[k, f]: feed transposed mats
    shared = {"s": np.ascontiguousarray(s32.T).astype(BF16NP),
              "s2": np.ascontiguousarray(s2.T).astype(BF16NP)}
    for c in CELLS:
        din = CELL_DIN[c]
        ga0, gw1, gw2 = _pad_w(weights[f"{c}_gate_W"], din, 2 * U)
        ca0, cw1, cw2 = _pad_w(weights[f"{c}_cand_W"], din, U)
        gb = np.zeros((128, 1), np.float32)
        gb[:, 0] = weights[f"{c}_gate_b"]
        cb = np.zeros((64, 1), np.float32)
        cb[:, 0] = weights[f"{c}_cand_b"]
        shared.update({
            f"{c}_gA0": ga0.astype(BF16NP),
            f"{c}_gW12": np.concatenate([gw1, gw2], axis=1).astype(BF16NP),
            f"{c}_cA0": ca0.astype(BF16NP),
            f"{c}_cW12": np.concatenate([cw1, cw2], axis=1).astype(BF16NP),
            f"{c}_gb": gb, f"{c}_cb": cb})
    shared["pW"] = np.ascontiguousarray(weights["proj_W"], np.float32).astype(BF16NP)
    shared["pb"] = np.asarray(weights["proj_b"], np.float32).reshape(1, 1)

    # inputs (T, B, N*DIN) -> per-core (T, DIN, AF) with node padding
    x = np.asarray(inputs, np.float32).reshape(T, B, N, DIN)
    per_core = []
    for c in range(NCORES):
        xc = x[:, c * BL:(c + 1) * BL]                  # (T, BL, N, DIN)
        xp = np.zeros((T, DIN, BL, NB), np.float32)
        xp[:, :, :, 0:N] = xc.transpose(0, 3, 1, 2)
        per_core.append(xp.reshape(T, DIN, AF).astype(BF16NP))
    return shared, per_core


def kernel(**inputs) -> np.ndarray:
    support = np.asarray(inputs["support"], np.float32)
    weights = {k: np.asarray(v, np.float32) for k, v in inputs.items()
               if k not in ("inputs", "support")}
    shared, per_core_x = _prep_inputs(inputs["inputs"], support, weights)

    nc = _build(T, HZ)
    if os.environ.get("DCRNN_TRACE"):
        _install_ntff_hook()
    in_maps = [dict(shared, x=per_core_x[c]) for c in range(NCORES)]
    res = run_bass_kernel_spmd(nc, in_maps, list(range(NCORES)),
                               trace=bool(os.environ.get("DCRNN_TRACE")))
    global LAST_RESULT
    LAST_RESULT = res
    if res.exec_time_ns is not None:
        print(f"HW exec time: {res.exec_time_ns} ns")
    outs = [np.asarray(res.results[c]["out"]).astype(np.float32)
            .reshape(HZ, BL, N) for c in range(NCORES)]
    return np.concatenate(outs, axis=1).astype(np.float32)


if __name__ == "__main__":
    sys.path.insert(0, "/root/problem")
    import reference
    ins = reference.setup_inputs()
    ins = {k: np.asarray(v) for k, v in ins.items()}
    exp = np.asarray(reference.reference(**ins))
    act = kernel(**ins)
    err = np.max(np.abs(act - exp)) / (np.abs(exp).max() + 1e-30)
    print("Relative error:", err)


# revision 15
# speedup vs baseline: 3.2201x; 1.2341x over previous
"""DCRNN (PEMS-BAY) Trainium2 Bass kernel, data-parallel over batch on 8 cores.

Transposeless formulation, all matmul inputs bf16 (PSUM accumulates fp32):
  gconv(X) = X@A0 + S@(X@W1) + S2@(X@2W2),  A0 = W0 - W2, S2 = S@S.
Per batch j (4 per half):
  down:  P_j = (X_Aslab_j)^T @ [W1|2W2]   -> B-layout P1|P2 in PSUM -> SBUF bf16
  up:    pre_j = A0^T @ X_j + sum_k P1_j[k,:]^T @ S[k,:] + P2_j[k,:]^T @ S2[k,:]
         (7 accumulating matmuls into one PSUM bank, A-layout [fout, 325])
  act:   sigmoid/tanh directly from PSUM with per-partition bias.
State tiles XH per cell/half: rows 0:64 = h, rows 64:64+din = x (A-layout,
cols = b*384 + n).  Decoder reuses encoder state tiles (no handoff copies).
"""
import sys
import os
import numpy as np

sys.path.insert(0, "/opt/trn_rl_repo")

import concourse.bass as bass  # noqa: E402
import concourse.mybir as mybir  # noqa: E402
import concourse.tile as tile  # noqa: E402
from concourse import bacc  # noqa: E402
from concourse.bass_utils import run_bass_kernel_spmd  # noqa: E402

import ml_dtypes  # noqa: E402

# problem constants
N = 325
B = 64
T = 12
HZ = 12
U = 64
DIN = 2
DOUT = 1
NCORES = 8
BL = B // NCORES          # 8 local batch
NB = N                    # node stride per batch (unpadded)
AF = BL * NB              # 2600 A-layout free width
NCH = [(0, 128), (128, 128), (256, 69)]   # node chunks (offset, len)
HB = BL // 2              # 4 batches per half
HAF = HB * NB             # 1300 A-cols per half

F32 = mybir.dt.float32
BF16 = mybir.dt.bfloat16
AFT = mybir.ActivationFunctionType
ALU = mybir.AluOpType
BF16NP = np.dtype(ml_dtypes.bfloat16)

CELLS = ["enc0", "enc1", "dec0", "dec1"]
CELL_DIN = {"enc0": DIN, "enc1": U, "dec0": DOUT, "dec1": U}

_BUILD_CACHE = {}
LAST_RESULT = None


def _install_ntff_hook():
    """Register the axon NTFF profiling hook if the image lacks antenv.axon_hooks."""
    import types
    import antenv
    if getattr(antenv, "axon_hooks", None) is not None:
        return
    m = types.ModuleType("antenv.axon_hooks")
    state = {"h": None}
    m.set_axon_ntff_profile_hook = lambda h: state.__setitem__("h", h)
    m.get_axon_ntff_profile_hook = lambda: state["h"]
    sys.modules["antenv.axon_hooks"] = m
    antenv.axon_hooks = m
    try:
        from trn_agent_boot.trn_boot import _ntff_profile_via_ctypes
        hook = _ntff_profile_via_ctypes("/opt/axon/libaxon_pjrt.so")
        if hook is not None:
            m.set_axon_ntff_profile_hook(hook)
    except Exception:
        pass


def _pad_w(w, din, fout):
    """(3F, fout) -> padded [128, fout] mats A0=W0-W2, W1, 2*W2.

    Padded row map: rows 0:64 <- h features (orig rows din:F),
    rows 64:64+din <- x features (orig rows 0:din). Others zero.
    """
    f = din + U
    w0, w1, w2 = w[0:f], w[f:2 * f], w[2 * f:3 * f]

    def pad(m):
        p = np.zeros((128, fout), np.float32)
        p[0:64] = m[din:f]
        p[64:64 + din] = m[0:din]
        return p

    return pad(w0 - w2), pad(w1), pad(2.0 * w2)


def _build(nsteps_enc, nsteps_dec):
    key = (nsteps_enc, nsteps_dec)
    if key in _BUILD_CACHE:
        return _BUILD_CACHE[key]

    nc = bacc.Bacc()
    # ---- DRAM params ----
    x_in = nc.declare_dram_parameter("x", [T, DIN, AF], BF16, isOutput=False)
    s_in = nc.declare_dram_parameter("s", [N, N], BF16, isOutput=False)
    s2_in = nc.declare_dram_parameter("s2", [N, N], BF16, isOutput=False)
    wparams = {}
    for c in CELLS:
        for nm, shp, dt_ in [("gA0", [128, 128], BF16), ("gW12", [128, 256], BF16),
                             ("cA0", [128, 64], BF16), ("cW12", [128, 128], BF16),
                             ("gb", [128, 1], F32), ("cb", [128, 1], F32)]:
            wparams[f"{c}_{nm}"] = nc.declare_dram_parameter(
                f"{c}_{nm}", shp, dt_, isOutput=False)
    wparams["pW"] = nc.declare_dram_parameter("pW", [64, 1], BF16, isOutput=False)
    wparams["pb"] = nc.declare_dram_parameter("pb", [1, 1], F32, isOutput=False)
    out_d = nc.declare_dram_parameter("out", [HZ, 1, BL, N], BF16, isOutput=True)

    with tile.TileContext(nc) as tc:
        with tc.tile_pool(name="const", bufs=1) as cp, \
             tc.tile_pool(name="state", bufs=1) as st, \
             tc.tile_pool(name="bp", bufs=2) as bp, \
             tc.tile_pool(name="pdp", bufs=4, space="PSUM") as pdp, \
             tc.tile_pool(name="pup", bufs=4, space="PSUM") as pup:

            # ---- constants to SBUF ----
            wt = {}
            for c in CELLS:
                for nm, shp in [("gA0", [128, 128]), ("gW12", [128, 256]),
                                ("cA0", [128, 64]), ("cW12", [128, 128])]:
                    wt[f"{c}_{nm}"] = cp.tile(shp, BF16, tag=f"{c}_{nm}",
                                              name=f"{c}_{nm}")
                wt[f"{c}_gb"] = cp.tile([128, 1], F32, tag=f"{c}_gb", name=f"{c}_gb")
                wt[f"{c}_cb"] = cp.tile([128, 1], F32, tag=f"{c}_cb", name=f"{c}_cb")
            wt["pW"] = cp.tile([64, 1], BF16, tag="pW", name="pW")
            wt["pb"] = cp.tile([1, 1], F32, tag="pb", name="pb")
            for k, t in wt.items():
                nc.sync.dma_start(out=t, in_=wparams[k][:])
            s_t, s2_t = [], []
            for ci, (c0, cl) in enumerate(NCH):
                stl = cp.tile([128, N], BF16, tag=f"s{ci}", name=f"s{ci}")
                nc.sync.dma_start(out=stl[0:cl, :], in_=s_in[c0:c0 + cl, :])
                s_t.append(stl)
                stl2 = cp.tile([128, N], BF16, tag=f"t{ci}", name=f"t{ci}")
                nc.sync.dma_start(out=stl2[0:cl, :], in_=s2_in[c0:c0 + cl, :])
                s2_t.append(stl2)

            # ---- state tiles ----
            xh = {}
            xr_h, ru_h, cc_h, u0_h = {}, {}, {}, {}
            for hf in range(2):
                for c in ["enc0", "enc1"]:
                    xh[(c, hf)] = st.tile([128, HAF], BF16, tag=f"xh_{c}_{hf}",
                                          name=f"xh_{c}_{hf}")
                xr_h[hf] = st.tile([128, HAF], BF16, tag=f"xr{hf}", name=f"xr{hf}")
                ru_h[hf] = st.tile([128, HAF], BF16, tag=f"ru{hf}", name=f"ru{hf}")
                cc_h[hf] = st.tile([64, HAF], BF16, tag=f"cc{hf}", name=f"cc{hf}")
                u0_h[hf] = st.tile([64, HAF], BF16, tag=f"u0{hf}", name=f"u0{hf}")

            for tl in ([xh[k] for k in xh]
                       + [xr_h[0], xr_h[1], ru_h[0], ru_h[1], cc_h[0], cc_h[1],
                          u0_h[0], u0_h[1]]):
                nc.vector.memset(tl[:, :].bitcast(F32), 0.0)
            tc.strict_bb_all_engine_barrier()

            evac_flip = [0]

            def evac(dst, src_ap):
                if evac_flip[0] % 2 == 0:
                    nc.scalar.copy(dst, src_ap)
                else:
                    nc.vector.tensor_copy(dst, src_ap)
                evac_flip[0] += 1

            def gconv(cell, src, pref, fout, hf, act_fn):
                """One gconv on one batch-half from A-layout src [128, HAF].
                act_fn(j, pu, row0) must consume pu[row0:row0+fout, 0:N]
                (A-layout preact for local batch j, bias NOT yet applied).
                fout=64 packs batch pairs: PB layout [P1 x4 | P2 x4] and one
                128-col stationary computes two batches per matmul."""
                w12 = wt[f"{cell}_{pref}W12"]
                a0 = wt[f"{cell}_{pref}A0"]
                w2 = 2 * fout
                per = 512 // w2          # batches per psum bank: gate 2, cand 4
                pbs = []
                for ci, (c0, cl) in enumerate(NCH):
                    pb = bp.tile([128, HB * w2], BF16, tag=f"pb{pref}{hf}_{ci}",
                                 name=f"pb{pref}{hf}_{ci}")
                    for g0 in range(0, HB, per):
                        js = list(range(g0, min(g0 + per, HB)))
                        pd = pdp.tile([128, 512], F32, tag="pd")
                        for idx, j in enumerate(js):
                            nc.tensor.matmul(
                                pd[0:cl, idx * w2:(idx + 1) * w2],
                                src[0:128, j * NB + c0: j * NB + c0 + cl],
                                w12[0:128, 0:w2],
                                start=(idx == 0), stop=(idx == len(js) - 1))
                        if fout == 128:
                            lo = g0 * w2
                            evac(pb[0:cl, lo:lo + 512], pd[0:cl, 0:512])
                        else:
                            # regroup [P1_j|P2_j] x4 -> [P1 x4 | P2 x4]
                            pdr = pd[0:cl, 0:512].rearrange(
                                "p (j t f) -> p t j f", j=HB, t=2)
                            for ti in range(2):
                                evac(pb[0:cl, ti * 256:(ti + 1) * 256].rearrange(
                                    "p (j f) -> p j f", j=HB),
                                    pdr[:, ti, :, :])
                    pbs.append(pb)
                if fout == 128:
                    for j in range(HB):
                        pu = pup.tile([128, 512], F32, tag="pu")
                        nc.tensor.matmul(pu[0:128, 0:N], a0[0:128, 0:128],
                                         src[0:128, j * NB: j * NB + N],
                                         start=True, stop=False)
                        for ci, (c0, cl) in enumerate(NCH):
                            nc.tensor.matmul(pu[0:128, 0:N],
                                             pbs[ci][0:cl, j * 256: j * 256 + 128],
                                             s_t[ci][0:cl, 0:N],
                                             start=False, stop=False)
                            nc.tensor.matmul(pu[0:128, 0:N],
                                             pbs[ci][0:cl, j * 256 + 128: j * 256 + 256],
                                             s2_t[ci][0:cl, 0:N],
                                             start=False, stop=(ci == 2))
                        act_fn(j, pu, 0)
                else:
                    for p0 in (0, 2):
                        pi = p0 // 2
                        pu = pup.tile([128, 512], F32, tag="pu")
                        nc.tensor.matmul(pu[0:64, 0:N], a0[0:128, 0:64],
                                         src[0:128, p0 * NB: p0 * NB + N],
                                         start=True, stop=False)
                        # start=True again: pending-zero marking is per
                        # partition, and this matmul covers rows 64:128
                        nc.tensor.matmul(pu[64:128, 0:N], a0[0:128, 0:64],
                                         src[0:128, (p0 + 1) * NB: (p0 + 1) * NB + N],
                                         start=True, stop=False)
                        for ci, (c0, cl) in enumerate(NCH):
                            nc.tensor.matmul(pu[0:128, 0:N],
                                             pbs[ci][0:cl, pi * 128: pi * 128 + 128],
                                             s_t[ci][0:cl, 0:N],
                                             start=False, stop=False)
                            nc.tensor.matmul(pu[0:128, 0:N],
                                             pbs[ci][0:cl, 256 + pi * 128: 384 + pi * 128],
                                             s2_t[ci][0:cl, 0:N],
                                             start=False, stop=(ci == 2))
                        act_fn(p0, pu, 0)
                        act_fn(p0 + 1, pu, 64)

            def cell(cname, hf, xh_t, nxt_xh):
                """DCGRU cell on one batch-half. x rows 64:64+din, h rows 0:64."""
                xr_t, ru_t, cc_t = xr_h[hf], ru_h[hf], cc_h[hf]
                gb = wt[f"{cname}_gb"]
                cb = wt[f"{cname}_cb"]

                def gate_act(j, pu, row0):
                    nc.scalar.activation(ru_t[0:128, j * NB: j * NB + N],
                                         pu[0:128, 0:N], AFT.Sigmoid,
                                         bias=gb[0:128, 0:1])

                def cand_act(j, pu, row0):
                    nc.scalar.activation(cc_t[0:64, j * NB: j * NB + N],
                                         pu[row0:row0 + 64, 0:N], AFT.Tanh,
                                         bias=cb[row0:row0 + 64, 0:1])

                gconv(cname, xh_t, "g", 128, hf, gate_act)
                nc.vector.tensor_tensor(xr_t[0:64, :], ru_t[0:64, :],
                                        xh_t[0:64, :], ALU.mult)
                # u lives at partitions 64:128; DVE tensor_tensor needs equal
                # SBUF base partitions, so stage it at base 0 first
                nc.vector.tensor_copy(u0_h[hf][0:64, :], ru_t[64:128, :])
                gconv(cname, xr_t, "c", 64, hf, cand_act)
                # h' = c + u*(h-c); ru rows 0:64 (r) are dead, reuse as tmp
                nc.vector.tensor_tensor(ru_t[0:64, :], xh_t[0:64, :],
                                        cc_t[0:64, :], ALU.subtract)
                nc.vector.tensor_tensor(ru_t[0:64, :], u0_h[hf][0:64, :],
                                        ru_t[0:64, :], ALU.mult)
                nc.vector.tensor_tensor(xh_t[0:64, :], cc_t[0:64, :],
                                        ru_t[0:64, :], ALU.add)
                if nxt_xh is not None:
                    nc.vector.tensor_copy(nxt_xh[64:128, :], xh_t[0:64, :])
                    nc.vector.tensor_copy(xr_t[64:128, :], xh_t[0:64, :])

            # ---- encoder ----
            xd = x_in[:]
            for t in range(nsteps_enc):
                for hf in range(2):
                    sl = slice(hf * HAF, (hf + 1) * HAF)
                    nc.sync.dma_start(out=xh[("enc0", hf)][64:64 + DIN, :],
                                      in_=xd[t, :, sl])
                    nc.sync.dma_start(out=xr_h[hf][64:64 + DIN, :],
                                      in_=xd[t, :, sl])
                for hf in range(2):
                    cell("enc0", hf, xh[("enc0", hf)], xh[("enc1", hf)])
                for hf in range(2):
                    cell("enc1", hf, xh[("enc1", hf)], None)

            # ---- decoder (reuses encoder state tiles; GO symbol = 0) ----
            for hf in range(2):
                nc.vector.memset(xh[("enc0", hf)][64:65, :].bitcast(F32), 0.0)
                nc.vector.memset(xr_h[hf][64:65, :].bitcast(F32), 0.0)

            for t in range(nsteps_dec):
                for hf in range(2):
                    cell("dec0", hf, xh[("enc0", hf)], xh[("enc1", hf)])
                for hf in range(2):
                    cell("dec1", hf, xh[("enc1", hf)], None)
                    h1 = xh[("enc1", hf)]
                    x0 = xh[("enc0", hf)]
                    for lo in range(0, HAF, 512):
                        w = min(512, HAF - lo)
                        pu = pup.tile([128, 512], F32, tag="pu")
                        nc.tensor.matmul(pu[0:1, 0:w], wt["pW"][0:64, 0:1],
                                         h1[0:64, lo:lo + w],
                                         start=True, stop=True)
                        nc.scalar.activation(x0[64:65, lo:lo + w],
                                             pu[0:1, 0:w], AFT.Identity,
                                             bias=wt["pb"][0:1, 0:1])
                    nc.vector.tensor_copy(xr_h[hf][64:65, :], x0[64:65, :])
                    ov = x0[64:65, :].rearrange("p (b n) -> p b n", b=HB)
                    nc.sync.dma_start(out=out_d[t][:, hf * HB:(hf + 1) * HB, :],
                                      in_=ov[:, :, 0:N])

    nc.finalize()
    _BUILD_CACHE[key] = nc
    return nc


def _prep_inputs(inputs, support, weights):
    """Host-side prep. Returns (shared_map, per_core_x list)."""
    s32 = np.asarray(support, np.float32)
    s2 = s32 @ s32
    # matmuls contract as sum_k M[k, m] * X[k, f]: feed transposed mats
    shared = {"s": np.ascontiguousarray(s32.T).astype(BF16NP),
              "s2": np.ascontiguousarray(s2.T).astype(BF16NP)}
    for c in CELLS:
        din = CELL_DIN[c]
        ga0, gw1, gw2 = _pad_w(weights[f"{c}_gate_W"], din, 2 * U)
        ca0, cw1, cw2 = _pad_w(weights[f"{c}_cand_W"], din, U)
        gb = np.zeros((128, 1), np.float32)
        gb[:, 0] = weights[f"{c}_gate_b"]
        cb = np.zeros((128, 1), np.float32)
        cb[0:64, 0] = weights[f"{c}_cand_b"]
        cb[64:128, 0] = weights[f"{c}_cand_b"]
        shared.update({
            f"{c}_gA0": ga0.astype(BF16NP),
            f"{c}_gW12": np.concatenate([gw1, gw2], axis=1).astype(BF16NP),
            f"{c}_cA0": ca0.astype(BF16NP),
            f"{c}_cW12": np.concatenate([cw1, cw2], axis=1).astype(BF16NP),
            f"{c}_gb": gb, f"{c}_cb": cb})
    shared["pW"] = np.ascontiguousarray(weights["proj_W"], np.float32).astype(BF16NP)
    shared["pb"] = np.asarray(weights["proj_b"], np.float32).reshape(1, 1)

    # inputs (T, B, N*DIN) -> per-core (T, DIN, AF)
    x = np.asarray(inputs, np.float32).reshape(T, B, N, DIN)
    per_core = []
    for c in range(NCORES):
        xc = x[:, c * BL:(c + 1) * BL]                  # (T, BL, N, DIN)
        xp = np.ascontiguousarray(xc.transpose(0, 3, 1, 2))
        per_core.append(xp.reshape(T, DIN, AF).astype(BF16NP))
    return shared, per_core


def kernel(**inputs) -> np.ndarray:
    support = np.asarray(inputs["support"], np.float32)
    weights = {k: np.asarray(v, np.float32) for k, v in inputs.items()
               if k not in ("inputs", "support")}
    shared, per_core_x = _prep_inputs(inputs["inputs"], support, weights)

    nc = _build(T, HZ)
    if os.environ.get("DCRNN_TRACE"):
        _install_ntff_hook()
    in_maps = [dict(shared, x=per_core_x[c]) for c in range(NCORES)]
    res = run_bass_kernel_spmd(nc, in_maps, list(range(NCORES)),
                               trace=bool(os.environ.get("DCRNN_TRACE")))
    global LAST_RESULT
    LAST_RESULT = res
    if res.exec_time_ns is not None:
        print(f"HW exec time: {res.exec_time_ns} ns")
    outs = [np.asarray(res.results[c]["out"]).astype(np.float32)
            .reshape(HZ, BL, N) for c in range(NCORES)]
    return np.concatenate(outs, axis=1).astype(np.float32)


if __name__ == "__main__":
    sys.path.insert(0, "/root/problem")
    import reference
    ins = reference.setup_inputs()
    ins = {k: np.asarray(v) for k, v in ins.items()}
    exp = np.asarray(reference.reference(**ins))
    act = kernel(**ins)
    err = np.max(np.abs(act - exp)) / (np.abs(exp).max() + 1e-30)
    print("Relative error:", err)
